# revision 11
# baseline (speedup 1.0000x reference)
"""Trainium2 Bass kernel for nn_CUSTOM_RNN_LAYER (gated RNN w/ RelaxedBernoulli gate).

Math (per time step t, batch row b):
    c_in  = tanh(x_t @ W_cin.T + b_cin)                      # precomputable for all t
    com   = [c, c_in]
    a     = relu(relu(relu(com@W1.T+b1)@W2.T+b2)@W3.T+b3)
    logit = a @ W4.T + b4
    alpha = sigmoid((logit + logistic_t) / TEMP)             # logistic precomputed on host
    gru   = GRUCell(c, h)  (gates r,z,n packed in Wih/Whh)
    h'    = h*(1-alpha) + alpha*gru  = h - alpha*(1-z)*(h - ng)
    n'    = n*(1-alpha) + 1
    c'    = (c*n*(1-alpha) + c_in) / n'
Outputs: (c_seq = all c_in, h_seq = all h', h_final = GRUCell(c_T, h_T))

Sharding: data-parallel over batch, 32 rows per core x 8 cores, weights
replicated, recurrence local per core (no collectives).

Layout strategy per core (batch-part = [batch(32) partitions, features free]):
  - All big matmuls stream the (replicated) weights as the MOVING operand in
    float32r (1 cycle/row at N>=256); the stationary operand is the transposed
    activation [K=128 chunk, M=32].
  - Activation transposes are done on TensorE (transpose-mode via identity),
    packed into shared PSUM banks, and copied to SBUF in one DVE op.
  - c_inT (feature-part tanh(xW)) is precomputed for all t and kept resident
    in SBUF; c_seq is written entirely during the precompute phase.
  - z-gate weight rows are pre-negated on host so one sigmoid yields (1-z).
"""

import sys

if "/opt/trn_rl_repo" not in sys.path:
    sys.path.insert(0, "/opt/trn_rl_repo")

import ml_dtypes
import numpy as np

import concourse.bass as bass
import concourse.bacc as bacc
import concourse.mybir as mybir
import concourse.tile as tile
from concourse.bass_utils import run_bass_kernel_spmd

B, T_FULL, I, H = 256, 512, 128, 256
NCORES = 8
BL = B // NCORES  # 32
H2 = 2 * H  # 512
H3 = 3 * H  # 768
TEMP = 0.1

F32 = mybir.dt.float32
BF16 = mybir.dt.bfloat16
AF = mybir.ActivationFunctionType
ALU = mybir.AluOpType


def mmr(nc, out, lhsT, rhs, start, stop):
    """matmul: out (+)= lhsT.T @ rhs (operands same dtype: bf16 loop, fp32 pre)"""
    nc.tensor.matmul(out, lhsT, rhs, start=start, stop=stop)


def build_nc(T, nz):
    """Build the Bass program. nz: dict of which biases are nonzero."""
    nc = bacc.Bacc()

    # ---------------- DRAM I/O ----------------
    x_d = nc.dram_tensor("x", [BL * T, I], F32, kind="ExternalInput")
    l10_d = nc.dram_tensor("l10", [BL, T], F32, kind="ExternalInput")
    l10n_d = nc.dram_tensor("l10n", [BL, T], F32, kind="ExternalInput")
    wcinT_d = nc.dram_tensor("wcinT", [I, H], F32, kind="ExternalInput")
    w1T_d = nc.dram_tensor("w1T", [4, 128, H2], BF16, kind="ExternalInput")
    w2T_d = nc.dram_tensor("w2T", [4, 128, H2], BF16, kind="ExternalInput")
    w3T_d = nc.dram_tensor("w3T", [4, 128, H2], BF16, kind="ExternalInput")
    wgiT_d = nc.dram_tensor("wgiT", [2, 128, H3], BF16, kind="ExternalInput")
    wghT_d = nc.dram_tensor("wghT", [2, 128, H3], BF16, kind="ExternalInput")
    w4T_d = nc.dram_tensor("w4T", [4, 128], BF16, kind="ExternalInput")
    bcin2_d = nc.dram_tensor("bcin2", [128, 2], F32, kind="ExternalInput")
    if nz["b1"]:
        b1_d = nc.dram_tensor("b1r", [1, H2], F32, kind="ExternalInput")
    if nz["b2"]:
        b2_d = nc.dram_tensor("b2r", [1, H2], F32, kind="ExternalInput")
    if nz["b3"]:
        b3_d = nc.dram_tensor("b3r", [1, H2], F32, kind="ExternalInput")
    if nz["bg"]:
        bg_d = nc.dram_tensor("bgr", [1, H3], F32, kind="ExternalInput")

    cseq_d = nc.dram_tensor("c_out", [BL, T, H], F32, kind="ExternalOutput")
    hseq_d = nc.dram_tensor("h_out", [BL, T, H], F32, kind="ExternalOutput")
    hfin_d = nc.dram_tensor("hf_out", [BL, H], F32, kind="ExternalOutput")

    eye_d = nc.inline_tensor(np.eye(128, dtype=np.float32), name="eye128")
    eyeb_d = nc.inline_tensor(np.eye(128, dtype=np.float32).astype(ml_dtypes.bfloat16), name="eyeb128")
    ones_d = nc.inline_tensor(np.ones((1, 128), dtype=np.float32), name="ones1")

    with tile.TileContext(nc) as tc:
        with (
            tc.tile_pool(name="const", bufs=1) as constp,
            tc.tile_pool(name="resident", bufs=1) as resp,
            tc.tile_pool(name="xin", bufs=3) as xinp,
            tc.tile_pool(name="xtbig", bufs=2) as xtbigp,
            tc.tile_pool(name="cinb", bufs=3) as cinbp,
            tc.tile_pool(name="state", bufs=2) as statep,
            tc.tile_pool(name="work", bufs=2) as workp,
            tc.tile_pool(name="small", bufs=3) as smallp,
            tc.tile_pool(name="ps", bufs=1, space="PSUM") as psp,
        ):
            # ------------- load constants / weights -------------
            eye_s = constp.tile([128, 128], F32)
            nc.sync.dma_start(eye_s[:], eye_d[:])
            eyeb_s = constp.tile([128, 128], BF16)
            nc.sync.dma_start(eyeb_s[:], eyeb_d[:])
            ones_s = constp.tile([1, 128], F32)
            nc.sync.dma_start(ones_s[:], ones_d[:])
            wcinT_s = constp.tile([128, H], F32)
            nc.sync.dma_start(wcinT_s[:], wcinT_d[:])
            w1T_s = constp.tile([128, 4, H2], BF16)
            nc.sync.dma_start(w1T_s[:], w1T_d.rearrange("k p n -> p k n"))
            w2T_s = constp.tile([128, 4, H2], BF16)
            nc.sync.dma_start(w2T_s[:], w2T_d.rearrange("k p n -> p k n"))
            w3T_s = constp.tile([128, 4, H2], BF16)
            nc.sync.dma_start(w3T_s[:], w3T_d.rearrange("k p n -> p k n"))
            wgiT_s = constp.tile([128, 2, H3], BF16)
            nc.sync.dma_start(wgiT_s[:], wgiT_d.rearrange("k p n -> p k n"))
            wghT_s = constp.tile([128, 2, H3], BF16)
            nc.sync.dma_start(wghT_s[:], wghT_d.rearrange("k p n -> p k n"))
            w4T_s = constp.tile([128, 4], BF16)
            nc.sync.dma_start(w4T_s[:], w4T_d.rearrange("k p -> p k"))
            l10_s = constp.tile([BL, T], F32)
            nc.sync.dma_start(l10_s[:], l10_d[:])
            l10n_s = constp.tile([BL, T], F32)
            nc.sync.dma_start(l10n_s[:], l10n_d[:])
            bcin2_s = constp.tile([128, 2], F32)
            nc.sync.dma_start(bcin2_s[:], bcin2_d[:])
            bias_rows = {}
            for key, d in [
                ("b1", b1_d if nz["b1"] else None),
                ("b2", b2_d if nz["b2"] else None),
                ("b3", b3_d if nz["b3"] else None),
                ("bg", bg_d if nz["bg"] else None),
            ]:
                if d is not None:
                    t_ = constp.tile(list(d.shape), F32)
                    nc.sync.dma_start(t_[:], d[:])
                    bias_rows[key] = t_

            # resident transposed c_in: [128, chunk(2), b(BL), t(T)]
            cinT_s = resp.tile([128, 2, BL, T], BF16)

            # ================= PRECOMPUTE =================
            # per batch row g: transpose X rows, compute c_inT (feature-part,
            # resident) and c_in (batch-part -> c_seq output DMA).
            JT = (T + 127) // 128  # 128-row t-tiles per batch row
            for g in range(BL):
                xT_sb = xtbigp.tile([128, JT, 128], F32)  # [i, j, t-sub]
                xpk = psp.tile([128, 512], F32, tag="pk1")
                for j in range(JT):
                    rows = min(128, T - j * 128)
                    x_t = xinp.tile([128, I], F32)
                    r0 = g * T + j * 128
                    nc.sync.dma_start(x_t[0:rows, :], x_d[r0 : r0 + rows, :])
                    nc.tensor.transpose(
                        xpk[:, j * 128 : j * 128 + rows],
                        x_t[0:rows, :],
                        eye_s[0:rows, 0:rows],
                    )
                xT_flat = xT_sb.rearrange("p a b -> p (a b)")
                nc.vector.tensor_copy(xT_flat[:, 0:T], xpk[:, 0:T])
                for k in range(2):
                    ps = psp.tile([128, 512], F32, tag="A")
                    mmr(
                        nc,
                        ps[:, 0:T],
                        wcinT_s[:, k * 128 : (k + 1) * 128],
                        xT_flat[:, 0:T],
                        True,
                        True,
                    )
                    nc.scalar.activation(
                        cinT_s[:, k, g, :],
                        ps[:, 0:T],
                        AF.Tanh,
                        bias=bcin2_s[:, k : k + 1] if nz["bcin"] else 0.0,
                    )
                # c_in batch-part (c_seq output): lhsT = xT tile, rhs = wcinT
                for j in range(JT):
                    rows = min(128, T - j * 128)
                    cps = psp.tile([128, H], F32, tag="B")
                    mmr(
                        nc,
                        cps[0:rows, :],
                        xT_sb[:, j, 0:rows],
                        wcinT_s[:],
                        True,
                        not nz["bcin"],
                    )
                    if nz["bcin"]:
                        # bias along free dim: ones-row matmul
                        bc_row = bcin2_s.rearrange("p k -> (k p)").unsqueeze(0)
                        mmr(nc, cps[0:rows, :], ones_s[:, 0:rows], bc_row, False, True)
                    cin_b = cinbp.tile([128, H], F32)
                    nc.scalar.activation(cin_b[0:rows, :], cps[0:rows, :], AF.Tanh)
                    nc.sync.dma_start(
                        cseq_d[g, j * 128 : j * 128 + rows, :], cin_b[0:rows, :]
                    )

            # ================= RECURRENT LOOP =================
            c_s = statep.tile([BL, H], F32, tag="c")
            h_s = statep.tile([BL, H], F32, tag="h")
            n_s = smallp.tile([BL, 1], F32, tag="n")
            nc.vector.memset(c_s[:], 0.0)
            nc.vector.memset(h_s[:], 0.0)
            nc.vector.memset(n_s[:], 0.0)

            def gates(sT_c0, sT_c1, sT_h0, sT_h1):
                """compute r|zc sigmoid tile and n-gate psum (inn|hn)."""
                D = psp.tile([BL, H2], F32, tag="D")
                mmr(nc, D[:], sT_c0, wgiT_s[:, 0, 0:H2], True, False)
                mmr(nc, D[:], sT_c1, wgiT_s[:, 1, 0:H2], False, False)
                mmr(nc, D[:], sT_h0, wghT_s[:, 0, 0:H2], False, False)
                last = not nz["bg"]
                mmr(nc, D[:], sT_h1, wghT_s[:, 1, 0:H2], False, last)
                if nz["bg"]:
                    mmr(nc, D[:], ones_s[:, 0:BL], bias_rows["bg"][:, 0:H2], False, True)
                E = psp.tile([BL, H2], F32, tag="E")
                mmr(nc, E[:, 0:H], sT_c0, wgiT_s[:, 0, H2:H3], True, False)
                last = not nz["bg"]
                mmr(nc, E[:, 0:H], sT_c1, wgiT_s[:, 1, H2:H3], False, last)
                if nz["bg"]:
                    mmr(nc, E[:, 0:H], ones_s[:, 0:BL], bias_rows["bg"][:, H2:H3], False, True)
                mmr(nc, E[:, H:H2], sT_h0, wghT_s[:, 0, H2:H3], True, False)
                mmr(nc, E[:, H:H2], sT_h1, wghT_s[:, 1, H2:H3], False, True)
                rz = workp.tile([BL, H2], F32, tag="rz")
                nc.scalar.activation(rz[:], D[:], AF.Sigmoid)
                # ngneg = tanh(-(inn + r*hn))
                t_r = workp.tile([BL, H], F32, tag="t_r")
                nc.vector.tensor_tensor(t_r[:], rz[:, 0:H], E[:, H:H2], ALU.mult)
                png = workp.tile([BL, H], F32, tag="png")
                nc.vector.tensor_tensor(png[:], t_r[:], E[:, 0:H], ALU.add)
                ngneg = workp.tile([BL, H], F32, tag="ngneg")
                nc.scalar.activation(ngneg[:], png[:], AF.Tanh, scale=-1.0)
                # e = zc * (h + ngneg)
                d_t = workp.tile([BL, H], F32, tag="d_t")
                nc.vector.tensor_tensor(d_t[:], h_s[:], ngneg[:], ALU.add)
                e_t = workp.tile([BL, H], F32, tag="e_t")
                nc.vector.tensor_tensor(e_t[:], rz[:, H:H2], d_t[:], ALU.mult)
                return e_t

            for t in range(T):
                # ---- pack transposes: cT(2) hT(2) + c_in batch-part(2) ----
                pk1 = psp.tile([128, 132], F32, tag="pk1")
                for k in range(2):
                    nc.tensor.transpose(
                        pk1[:, k * 32 : (k + 1) * 32],
                        c_s[:, k * 128 : (k + 1) * 128],
                        eye_s[0:BL, 0:BL],
                    )
                for k in range(2):
                    nc.tensor.transpose(
                        pk1[:, 64 + k * 32 : 64 + (k + 1) * 32],
                        h_s[:, k * 128 : (k + 1) * 128],
                        eye_s[0:BL, 0:BL],
                    )
                pk1b = psp.tile([BL, H], BF16, tag="pk1b")
                for k in range(2):
                    nc.tensor.transpose(
                        pk1b[:, k * 128 : (k + 1) * 128],
                        cinT_s[:, k, :, t],
                        eyeb_s[:],
                    )
                sT = workp.tile([128, 128], BF16, tag="sT")
                nc.vector.tensor_copy(sT[:], pk1[:, 0:128])
                cin_bp = pk1b  # [BL, H] batch-part c_in (PSUM, bf16)

                # ---- MLP layer 1 ----
                A = psp.tile([BL, H2], F32, tag="A")
                mmr(nc, A[:], sT[:, 0:32], w1T_s[:, 0, :], True, False)
                mmr(nc, A[:], sT[:, 32:64], w1T_s[:, 1, :], False, False)
                mmr(nc, A[:], cinT_s[:, 0, :, t], w1T_s[:, 2, :], False, False)
                last = not nz["b1"]
                mmr(nc, A[:], cinT_s[:, 1, :, t], w1T_s[:, 3, :], False, last)
                if nz["b1"]:
                    mmr(nc, A[:], ones_s[:, 0:BL], bias_rows["b1"][:], False, True)
                a1 = workp.tile([BL, H2], BF16, tag="a1")
                nc.scalar.activation(a1[:], A[:], AF.Relu)
                pk2 = psp.tile([128, 128], BF16, tag="pkA")
                for j in range(4):
                    nc.tensor.transpose(
                        pk2[:, j * 32 : (j + 1) * 32],
                        a1[:, j * 128 : (j + 1) * 128],
                        eyeb_s[0:BL, 0:BL],
                    )
                a1T = workp.tile([128, 128], BF16, tag="a1T")
                nc.vector.tensor_copy(a1T[:], pk2[:, 0:128])

                # ---- gates (overlap with MLP on other engines) ----
                e_t = gates(sT[:, 0:32], sT[:, 32:64], sT[:, 64:96], sT[:, 96:128])

                # ---- MLP layer 2 ----
                Bp = psp.tile([BL, H2], F32, tag="B")
                for j in range(4):
                    st = j == 0
                    last = (j == 3) and not nz["b2"]
                    mmr(nc, Bp[:], a1T[:, j * 32 : (j + 1) * 32], w2T_s[:, j, :], st, last)
                if nz["b2"]:
                    mmr(nc, Bp[:], ones_s[:, 0:BL], bias_rows["b2"][:], False, True)
                a2 = workp.tile([BL, H2], BF16, tag="a2")
                nc.scalar.activation(a2[:], Bp[:], AF.Relu)
                pk3 = psp.tile([128, 128], BF16, tag="pkA")
                for j in range(4):
                    nc.tensor.transpose(
                        pk3[:, j * 32 : (j + 1) * 32],
                        a2[:, j * 128 : (j + 1) * 128],
                        eyeb_s[0:BL, 0:BL],
                    )
                a2T = workp.tile([128, 128], BF16, tag="a2T")
                nc.vector.tensor_copy(a2T[:], pk3[:, 0:128])

                # ---- MLP layer 3 + logit ----
                Cp = psp.tile([BL, H2], F32, tag="C")
                for j in range(4):
                    st = j == 0
                    last = (j == 3) and not nz["b3"]
                    mmr(nc, Cp[:], a2T[:, j * 32 : (j + 1) * 32], w3T_s[:, j, :], st, last)
                if nz["b3"]:
                    mmr(nc, Cp[:], ones_s[:, 0:BL], bias_rows["b3"][:], False, True)
                a3 = workp.tile([BL, H2], BF16, tag="a3")
                nc.scalar.activation(a3[:], Cp[:], AF.Relu)
                pk4 = psp.tile([128, 128], BF16, tag="pkA")
                for j in range(4):
                    nc.tensor.transpose(
                        pk4[:, j * 32 : (j + 1) * 32],
                        a3[:, j * 128 : (j + 1) * 128],
                        eyeb_s[0:BL, 0:BL],
                    )
                a3T = workp.tile([128, 128], BF16, tag="a3T")
                nc.vector.tensor_copy(a3T[:], pk4[:, 0:128])
                logit = pk1[0:BL, 128:129]
                for j in range(4):
                    mmr(nc, logit, a3T[:, j * 32 : (j + 1) * 32],
                        w4T_s[:, j : j + 1], j == 0, j == 3)
                # alpha = sigmoid((logit + b4 + logistic)/TEMP); l10 has (b4+logistic)/TEMP
                alpha = smallp.tile([BL, 1], F32, tag="alpha")
                nc.scalar.activation(
                    alpha[:], logit, AF.Sigmoid, bias=l10_s[:, t : t + 1], scale=10.0
                )
                alpha_c = smallp.tile([BL, 1], F32, tag="alphac")
                nc.scalar.activation(
                    alpha_c[:], logit, AF.Sigmoid, bias=l10n_s[:, t : t + 1],
                    scale=-10.0,
                )

                # ---- state updates ----
                # h' = h - alpha * e
                f_t = workp.tile([BL, H], F32, tag="f_t")
                nc.vector.tensor_scalar(
                    f_t[:], e_t[:], alpha[:], None, ALU.mult
                )
                h_new = statep.tile([BL, H], F32, tag="h")
                nc.vector.tensor_tensor(h_new[:], h_s[:], f_t[:], ALU.subtract)
                nc.sync.dma_start(hseq_d[:, t, :], h_new[:])
                # n' = n*(1-alpha) + 1 ; c' = (c*w + c_in) / n'
                w_t = smallp.tile([BL, 1], F32, tag="w_t")
                nc.vector.tensor_tensor(w_t[:], n_s[:], alpha_c[:], ALU.mult)
                n_new = smallp.tile([BL, 1], F32, tag="n")
                nc.vector.tensor_scalar(n_new[:], w_t[:], 1.0, None, ALU.add)
                rn = smallp.tile([BL, 1], F32, tag="rn")
                nc.vector.reciprocal(rn[:], n_new[:])
                m3 = workp.tile([BL, H], F32, tag="m3")
                nc.vector.tensor_scalar(m3[:], c_s[:], w_t[:], None, ALU.mult)
                a3c = workp.tile([BL, H], F32, tag="a3c")
                nc.vector.tensor_tensor(a3c[:], m3[:], cin_bp, ALU.add)
                c_new = statep.tile([BL, H], F32, tag="c")
                nc.vector.tensor_scalar(c_new[:], a3c[:], rn[:], None, ALU.mult)

                c_s, h_s, n_s = c_new, h_new, n_new

            # ================= EPILOGUE: h_final =================
            pk1 = psp.tile([128, 128], F32, tag="pk1")
            for k in range(2):
                nc.tensor.transpose(
                    pk1[:, k * 32 : (k + 1) * 32],
                    c_s[:, k * 128 : (k + 1) * 128],
                    eye_s[0:BL, 0:BL],
                )
            for k in range(2):
                nc.tensor.transpose(
                    pk1[:, 64 + k * 32 : 64 + (k + 1) * 32],
                    h_s[:, k * 128 : (k + 1) * 128],
                    eye_s[0:BL, 0:BL],
                )
            sT = workp.tile([128, 128], BF16, tag="sT")
            nc.vector.tensor_copy(sT[:], pk1[:, 0:128])
            e_t = gates(sT[:, 0:32], sT[:, 32:64], sT[:, 64:96], sT[:, 96:128])
            h_fin = statep.tile([BL, H], F32, tag="hfin")
            nc.vector.tensor_tensor(h_fin[:], h_s[:], e_t[:], ALU.subtract)
            nc.sync.dma_start(hfin_d[:], h_fin[:])

    nc.compile()
    return nc


def _host_prep(input, noise, W_cin, b_cin, W1, b1, W2, b2, W3, b3, W4, b4,
               Wih, Whh, bih, bhh, T):
    f = np.float32
    inp = np.asarray(input, f)
    u = np.asarray(noise, f).reshape(-1, B)[:T]
    logistic = np.log(u) - np.log1p(-u)
    b4v = float(np.asarray(b4, f).reshape(-1)[0])
    l10_full = ((logistic + b4v) / TEMP).astype(f)  # [T, B]

    Wg_i = np.array(Wih, f)
    Wg_h = np.array(Whh, f)
    Wg_i[H : 2 * H] *= -1.0  # negate z rows -> sigmoid gives (1-z)
    Wg_h[H : 2 * H] *= -1.0
    bg = (np.asarray(bih, f) + np.asarray(bhh, f)).copy()
    bg[H : 2 * H] *= -1.0

    shared = {
        "wcinT": np.ascontiguousarray(np.asarray(W_cin, f).T),
        "w1T": np.ascontiguousarray(np.asarray(W1, f).T.reshape(4, 128, H2)).astype(ml_dtypes.bfloat16),
        "w2T": np.ascontiguousarray(np.asarray(W2, f).T.reshape(4, 128, H2)).astype(ml_dtypes.bfloat16),
        "w3T": np.ascontiguousarray(np.asarray(W3, f).T.reshape(4, 128, H2)).astype(ml_dtypes.bfloat16),
        "wgiT": np.ascontiguousarray(Wg_i.T.reshape(2, 128, H3)).astype(ml_dtypes.bfloat16),
        "wghT": np.ascontiguousarray(Wg_h.T.reshape(2, 128, H3)).astype(ml_dtypes.bfloat16),
        "w4T": np.ascontiguousarray(np.asarray(W4, f).reshape(H2).reshape(4, 128)).astype(ml_dtypes.bfloat16),
        "bcin2": np.ascontiguousarray(np.asarray(b_cin, f).reshape(2, 128).T),
            }
    nz = {
        "bcin": bool(np.any(b_cin)),
        "b1": bool(np.any(b1)),
        "b2": bool(np.any(b2)),
        "b3": bool(np.any(b3)),
        "bg": bool(np.any(bg)),
    }
    if nz["b1"]:
        shared["b1r"] = np.asarray(b1, f).reshape(1, H2)
    if nz["b2"]:
        shared["b2r"] = np.asarray(b2, f).reshape(1, H2)
    if nz["b3"]:
        shared["b3r"] = np.asarray(b3, f).reshape(1, H2)
    if nz["bg"]:
        shared["bgr"] = bg.reshape(1, H3)

    in_maps = []
    for ci in range(NCORES):
        b0 = ci * BL
        m = dict(shared)
        m["x"] = np.ascontiguousarray(
            inp[b0 : b0 + BL, :T, :].reshape(BL * T, I)
        )
        m["l10"] = np.ascontiguousarray(l10_full[:T, b0 : b0 + BL].T)
        m["l10n"] = np.ascontiguousarray(-l10_full[:T, b0 : b0 + BL].T)
        in_maps.append(m)
    return in_maps, nz


def run(inputs, T=T_FULL, trace=False):
    in_maps, nz = _host_prep(T=T, **inputs)
    nc = build_nc(T, nz)
    res = run_bass_kernel_spmd(nc, in_maps, list(range(NCORES)), trace=trace)
    rs = res.results
    c = np.concatenate([rs[i]["c_out"] for i in range(NCORES)], axis=0)
    h = np.concatenate([rs[i]["h_out"] for i in range(NCORES)], axis=0)
    hf = np.concatenate([rs[i]["hf_out"] for i in range(NCORES)], axis=0)
    return (c, h, hf), res


def kernel(**inputs):
    out, _ = run(inputs)
    return out
